# revision 82
# baseline (speedup 1.0000x reference)
"""Multi-head dot-product attention (causal) on 8 TRN2 NeuronCores.

Sharding (Megatron-style per hint): batch (2) x head-groups (4 of 4 heads)
= 8 cores. Each core: q/k/v projections for its 4 heads, causal attention,
partial output projection Y_c = sum_h O_h @ Wo_h. Host sums the 4 partials
per batch (the "all-reduce") in f32.

All four projections (Q/K/V and the output projection) run as fp8e4m3
DoubleRow matmuls (K=256 per pass) with 3-term error compensation:
A ~ A8+Ar, B ~ B8+Br, computing A8@B8 + A8@Br + Ar@B8 in one f32 PSUM
accumulation -- bf16-level accuracy at 0.75x the bf16 PE cost. Weights
are pre-scaled by 256 (and the normalized attention output by 1/16) to
keep all fp8 operands clear of e4m3 subnormals and below e4m3 max; the
scales are folded into the softmax exp scale and a final 1/4096
tensor_scalar on the y copy. X/W splits happen on the host; the
attention-output split runs on DVE (copy + subtract per head).
Attention itself (QK/AV) stays bf16. Rel err vs f32 ref ~4e-3 (tol 2e-2).

Single fused instruction stream, engine roles:
  PE:   warmup (p-state ramp), Q/KT/V projections, QK^T, diag-mask bias
        matmuls, AV, output projection. One continuous stream; filler
        matmuls from neighboring phases cover every dependency gap.
  ACT:  exp (softmax numerator, fused scale), KT/Q psum->sbuf copies.
  DVE:  half the denominator accumulation, V/Y psum copies, reciprocal,
        final O normalization (reads O straight from PSUM).
  Pool: other half of the denominator accumulation, cross-partition sum
        (partition_all_reduce).
  SP:   all load DMAs, batched into few large transfers (HWDGE config
        serializes globally at ~630ns/DMA, so fewer+bigger is faster),
        y stores.

Schedule: warmup | A: Q(0) proj (DMA-paced) | B01: KT/V for s<512 |
BC0: attention(t-tile 0) with KT/V(s>=512) interleaved as fillers |
C1: attention(1) with Q(2), Q(3), YO(0) fillers | C2: attention(2) with
YO(1) | C3: attention(3) with YO(2), 4-deep QK run-ahead | coda: YO(3).
Per-head softmax normalization chains are deferred into the next head so
the PE never waits on them. Causal masking: diagonal QK/AV matmuls are
range-restricted; the 128x128 triangle gets a -1e10 bias via a tiny
identity x pattern matmul accumulated onto the logits.
"""
import math
from collections import deque

import numpy as np

import concourse.bass as bass
import concourse.bass_isa as bass_isa
import concourse.mybir as mybir
import concourse.tile as tile
from concourse import bacc
from concourse import bass_utils
from concourse.masks import make_identity

f32 = mybir.dt.float32
bf16 = mybir.dt.bfloat16
fp8 = mybir.dt.float8e4
DR = mybir.MatmulPerfMode.DoubleRow
AF = mybir.ActivationFunctionType

# Problem shape (hardcoded per contract)
B, T, S, E, N, D = 2, 2048, 2048, 2048, 16, 128
N_CORES = 8
HL = 4              # heads per core
P = 128             # partitions
HD = HL * D         # 512
NE = E // P         # 16 contraction chunks
TT = 512            # t tile
NTT = T // TT       # 4
SB = 256            # phase-B s tile
NST = S // SB       # 8
NCH = 4             # e-chunks per DMA chunk tile (wq/wk/wv/xq)
SCALE = 1.0 / math.sqrt(D)
WS = 256.0          # fp8 weight pre-scale (keeps W out of e4m3 subnormals);
                    # q,k,v come out of the projections scaled by WS: folded
                    # into the exp scale (QK) and a host-side Wo/WS (AV path)
SCALE2 = SCALE / (WS * WS)
OS = 16.0           # onorm scale-down (short-prefix rows reach ~940/OS)
YSCALE = OS / (WS * WS)

MM_LABELS = {}


def build_nc():
    nc = bacc.Bacc("TRN2", target_bir_lowering=False, debug=False)

    def mm(label, *args, **kw):
        r = nc.tensor.matmul(*args, **kw)
        MM_LABELS[r.ins.name] = label
        return r

    # DRAM tensors; all host-packed so every load is a contiguous slice.
    xq_d = {s: nc.dram_tensor(f"xq{s}", [P, NE, T], fp8, kind="ExternalInput")
            for s in "8r"}
    xkv_d = {s: nc.dram_tensor(f"xkv{s}", [P, NE, S], fp8, kind="ExternalInput")
             for s in "8r"}
    wq_d = {s: nc.dram_tensor(f"wq{s}", [P, NE, HD], fp8, kind="ExternalInput")
            for s in "8r"}
    wk_d = {s: nc.dram_tensor(f"wk{s}", [P, NE, HD], fp8, kind="ExternalInput")
            for s in "8r"}
    wv_d = {s: nc.dram_tensor(f"wv{s}", [P, NE, HD], fp8, kind="ExternalInput")
            for s in "8r"}
    wo_d = {s: nc.dram_tensor(f"wo{s}", [P, HL, E], fp8, kind="ExternalInput")
            for s in "8r"}
    y_d = nc.dram_tensor("y", [P, T // P, E], bf16, kind="ExternalOutput")

    with tile.TileContext(nc) as tc:
        with tc.tile_pool(name="persist", bufs=1) as persist:
            kt_all = persist.tile([P, HL, S], bf16)        # K^T [d, h, s]
            wup = persist.tile([P, P], bf16)               # warmup operand:
            # cheap memset, then the p-state warmup matmuls read it
            # (result discarded)
            v_all = persist.tile([P, S // P, HD], bf16)    # V [s-in-blk, blk, hd]
            wo_all = {s: persist.tile([P, HL, E], fp8, name=f"wo{s}a")
                      for s in "8r"}                   # Wo*WS fp8 [d, h, e]
            tri = persist.tile([P, P], bf16)               # -1e10 strict lower tri
            ident = persist.tile([P, P], bf16)

            nc.gpsimd.memset(wup[:], 1.0)

            with tc.tile_pool(name="init", bufs=1) as initp:
                scr = initp.tile([P, P], f32)
                nc.gpsimd.memset(scr[:], 0.0)
                # keep 0 where tj - si >= 0, else fill -1e10
                nc.gpsimd.affine_select(
                    out=scr[:], in_=scr[:],
                    compare_op=mybir.AluOpType.is_ge,
                    fill=-1e10, base=0,
                    pattern=[[1, P]], channel_multiplier=-1,
                )
                with nc.allow_low_precision(reason="-1e10 mask bias to bf16"):
                    nc.vector.tensor_copy(tri[:], scr[:])
                idf = initp.tile([P, P], f32)
                make_identity(nc, idf[:])
                with nc.allow_low_precision(reason="identity to bf16"):
                    nc.vector.tensor_copy(ident[:], idf[:])

            # PE p-state warmup: one long accumulation group of dummy
            # matmuls keeps the PE continuously busy from ~1.5us so the
            # clock is fully ramped before the first real projection.
            NWU = 36

            # ---- long-lived ring pools ----
            wqp_cm = tc.tile_pool(name="wqp", bufs=1)
            wqp = wqp_cm.__enter__()
            wq_c = {s: [wqp.tile([P, NCH, HD], fp8, name=f"wq{s}{c}")
                        for c in range(NCH)] for s in "8r"}
            xqp_cm = tc.tile_pool(name="xqp", bufs=8)
            xqp = xqp_cm.__enter__()
            qtp_cm = tc.tile_pool(name="qtp", bufs=3)
            qtp = qtp_cm.__enter__()
            onp_cm = tc.tile_pool(name="onp", bufs=3)
            onp = onp_cm.__enter__()
            epp_cm = tc.tile_pool(name="epp", bufs=6)
            epp = epp_cm.__enter__()
            esp_cm = tc.tile_pool(name="esp", bufs=2)
            esp = esp_cm.__enter__()
            rbp_cm = tc.tile_pool(name="rbp", bufs=2)
            rbp = rbp_cm.__enter__()
            yp_cm = tc.tile_pool(name="yp", bufs=2)
            yp = yp_cm.__enter__()
            psl_cm = tc.tile_pool(name="psl", bufs=2, space="PSUM")
            psl = psl_cm.__enter__()
            pso_cm = tc.tile_pool(name="pso", bufs=2, space="PSUM")
            pso = pso_cm.__enter__()

            # phase-B-scoped pools
            wkvp_cm = tc.tile_pool(name="wkvp", bufs=1)
            wkvp = wkvp_cm.__enter__()
            wk_c = {s: [wkvp.tile([P, NCH, HD], fp8, name=f"wk{s}{c}")
                        for c in range(NCH)] for s in "8r"}
            wv_c = {s: [wkvp.tile([P, NCH, HD], fp8, name=f"wv{s}{c}")
                        for c in range(NCH)] for s in "8r"}
            xkvp_cm = tc.tile_pool(name="xkvp", bufs=2)
            xkvp = xkvp_cm.__enter__()
            pskt = psv = None  # PSUM pools opened after phase A

            xq_tiles = {}   # tt -> list of chunk tiles

            def load_xq(tt, chunks=range(NCH), terms="8r"):
                tiles = xq_tiles.setdefault(
                    tt, {s: [None] * NCH for s in "8r"})
                for s in terms:
                    for c in chunks:
                        if tiles[s][c] is not None:
                            continue
                        t = xqp.tile([P, NCH, TT], fp8, tag=f"xq{s}",
                                     name=f"xq{s}{tt}_{c}")
                        nc.sync.dma_start(
                            t[:], xq_d[s][:, c * NCH:(c + 1) * NCH,
                                          tt * TT:(tt + 1) * TT])
                        tiles[s][c] = t

            xkv_tiles = {}

            def load_xkv(st):
                if st in xkv_tiles:
                    return
                pair = {}
                for s in "8r":
                    t = xkvp.tile([P, NE, SB], fp8, tag=f"xkv{s}",
                                  name=f"xkv{s}{st}")
                    nc.sync.dma_start(t[:],
                                      xkv_d[s][:, :, st * SB:(st + 1) * SB])
                    pair[s] = t
                xkv_tiles[st] = pair

            def load_w(dst_chunks, src, chunks=range(NCH), terms="8r"):
                for s in terms:
                    for c in chunks:
                        nc.sync.dma_start(dst_chunks[s][c][:],
                                          src[s][:, c * NCH:(c + 1) * NCH, :])

            qt_tiles = {}

            # ---------------- Phase B work generator ----------------
            def b_st(st):
                """KT/V accumulation for one s tile; yields after each matmul."""
                if st + 1 < NST:
                    load_xkv(st + 1)
                xt8 = xkv_tiles[st]["8"]
                xtr = xkv_tiles[st]["r"]

                def kt_group(h):
                    ps = pskt.tile([P, SB], f32, tag="pskt", name=f"psKT{st}_{h}")
                    terms = ((wk_c["8"], xt8), (wk_c["r"], xt8),
                             (wk_c["8"], xtr))
                    for it, (wc, xc) in enumerate(terms):
                        for ep in range(NE // 2):
                            e0 = 2 * ep
                            c, ce = divmod(e0, NCH)
                            mm("KT", ps[:],
                               wc[c][:, ce:ce + 2, h * D:(h + 1) * D],
                               xc[:, e0:e0 + 2, :],
                               start=(it == 0 and ep == 0),
                               stop=(it == 2 and ep == NE // 2 - 1),
                               perf_mode=DR)
                            yield
                    with nc.allow_low_precision(reason="K^T stored bf16"):
                        nc.scalar.activation(
                            kt_all[:, h, st * SB:(st + 1) * SB], ps[:], AF.Copy)

                def v_group(j):
                    ps = psv.tile([P, HD], f32, tag="psv", name=f"psV{st}_{j}")
                    terms = ((xt8, wv_c["8"]), (xt8, wv_c["r"]),
                             (xtr, wv_c["8"]))
                    for it, (xc, wc) in enumerate(terms):
                        for ep in range(NE // 2):
                            e0 = 2 * ep
                            c, ce = divmod(e0, NCH)
                            mm("V", ps[:],
                               xc[:, e0:e0 + 2, j * P:(j + 1) * P],
                               wc[c][:, ce:ce + 2, :],
                               start=(it == 0 and ep == 0),
                               stop=(it == 2 and ep == NE // 2 - 1),
                               perf_mode=DR)
                            yield
                    with nc.allow_low_precision(reason="V stored bf16"):
                        nc.vector.tensor_copy(v_all[:, st * 2 + j, :], ps[:])

                # order: KTh0 Vj0 KTh1 KTh2 Vj1 KTh3 (rolling psum drains)
                for g in (kt_group(0), v_group(0), kt_group(1), kt_group(2),
                          v_group(1), kt_group(3)):
                    yield from g

            def b_work(sts):
                for st in sts:
                    yield from b_st(st)

            # deferred per-head normalization chains
            pending = deque()

            def drain(n=1):
                for _ in range(n):
                    while pending:
                        try:
                            next(pending[0])
                            return
                        except StopIteration:
                            pending.popleft()

            def flush_pending():
                while pending:
                    drain()

            def tail_gen(tt, h, psO, esumA, esumB, onorm):
                nc.vector.tensor_add(esumA[:], esumA[:], esumB[:])
                yield
                Rb = rbp.tile([P, TT], f32, tag="rb", name=f"rb{tt}_{h}")
                nc.gpsimd.partition_all_reduce(
                    Rb[:], esumA[:], channels=P, reduce_op=bass_isa.ReduceOp.add)
                yield
                rec = rbp.tile([P, TT], bf16, tag="rec", name=f"rec{tt}_{h}")
                with nc.allow_low_precision(reason="1/R feeds a bf16 matmul"):
                    nc.vector.reciprocal(rec[:], Rb[:])
                yield
                on8, onr = onorm
                on_bf = rbp.tile([P, TT], bf16, tag="onbf", name=f"ob{tt}_{h}")
                with nc.allow_low_precision(reason="normalized O is bf16"):
                    nc.vector.scalar_tensor_tensor(
                        on_bf[:], psO[:], 1.0 / OS, rec[:],
                        op0=mybir.AluOpType.mult, op1=mybir.AluOpType.mult)
                yield
                with nc.allow_low_precision(reason="fp8 split of O"):
                    nc.vector.tensor_copy(on8[:, h, :], on_bf[:])
                yield
                with nc.allow_low_precision(reason="fp8 residual of O"):
                    nc.vector.tensor_sub(onr[:, h, :], on_bf[:],
                                         on8[:, h, :])
                yield

            # ---------------- filler generators ----------------
            def q_work(tt, nbanks=2):
                """Q projection for tile tt; run during tile tt-1. With
                nbanks=4 (phase A: spare banks) all heads accumulate in one
                sweep so each xq chunk is fully consumed on arrival."""
                with tc.tile_pool(name=f"psq{tt}", bufs=nbanks,
                                  space="PSUM") as psq:
                    qt = qtp.tile([P, HL, TT], bf16, tag="qt", name=f"qt{tt}")
                    hper = nbanks
                    xq8, xqr = xq_tiles[tt]["8"], xq_tiles[tt]["r"]
                    terms = ((wq_c["8"], xq8), (wq_c["r"], xq8),
                             (wq_c["8"], xqr))
                    for sweep in range(HL // hper):
                        hs = tuple(range(sweep * hper, (sweep + 1) * hper))
                        ps = [psq.tile([P, TT], f32, tag="psq", name=f"psQ{tt}_{h}")
                              for h in hs]
                        for it, (wc, xc) in enumerate(terms):
                            for ep in range(NE // 2):
                                e0 = 2 * ep
                                c, ce = divmod(e0, NCH)
                                for i, h in enumerate(hs):
                                    mm("Q", ps[i][:],
                                       wc[c][:, ce:ce + 2, h * D:(h + 1) * D],
                                       xc[c][:, ce:ce + 2, :],
                                       start=(it == 0 and ep == 0),
                                       stop=(it == 2 and ep == NE // 2 - 1),
                                       perf_mode=DR)
                                    yield
                        with nc.allow_low_precision(reason="Q stored bf16"):
                            for i, h in enumerate(hs):
                                nc.scalar.activation(qt[:, h, :], ps[i][:],
                                                     AF.Copy)
                    qt_tiles[tt] = qt

            def yo_work(tt, psys):
                """Output projection for tile tt; run during tile tt+1.
                psum->sbuf copies on DVE (ACT is kept exp-only); one batched
                store per e-chunk."""
                on8, onr8 = on_tiles[tt]
                terms = ((on8, wo_all["8"]), (on8, wo_all["r"]),
                         (onr8, wo_all["8"]))
                nchunk = 0
                for et in range(E // TT):
                    ysb = yp.tile([P, TT // P, TT], bf16, tag="ysb",
                                  name=f"ysb{tt}_{et}")
                    for j in range(TT // P):
                        psy = psys[nchunk % len(psys)]
                        nchunk += 1
                        psY = psy.tile([P, TT], f32, tag="psy", name="psY")
                        for it, (oc, wc) in enumerate(terms):
                            for hp in (0, 2):
                                mm("YO", psY[:],
                                   oc[:, hp:hp + 2, j * P:(j + 1) * P],
                                   wc[:, hp:hp + 2, et * TT:(et + 1) * TT],
                                   start=(it == 0 and hp == 0),
                                   stop=(it == 2 and hp == 2),
                                   perf_mode=DR)
                                yield
                        with nc.allow_low_precision(reason="y partial bf16"):
                            if tt == NTT - 1 and nchunk % 2 == 0:
                                # coda: ACT is idle; a scaled Copy halves the
                                # final DVE mul chain before the last stores
                                nc.scalar.activation(ysb[:, j, :], psY[:],
                                                     AF.Copy, scale=YSCALE)
                            else:
                                nc.vector.tensor_scalar_mul(ysb[:, j, :],
                                                            psY[:], YSCALE)
                        if tt == NTT - 1 and et == E // TT - 1:
                            eng = nc.sync if j % 2 == 0 else nc.scalar
                            eng.dma_start(
                                y_d[:, tt * 4 + j, et * TT:(et + 1) * TT],
                                ysb[:, j, :])
                    if not (tt == NTT - 1 and et == E // TT - 1):
                        nc.sync.dma_start(
                            y_d[:, tt * 4:tt * 4 + 4, et * TT:(et + 1) * TT],
                            ysb[:])

            on_tiles = {}

            # ---------------- attention ----------------
            def attention(tt, filler, rate, psls=None):
                nsb = (tt + 1) * (TT // P)
                onorm = (onp.tile([P, HL, TT], fp8, tag="on8", name=f"on8{tt}"),
                         onp.tile([P, HL, TT], fp8, tag="onr", name=f"onr{tt}"))
                on_tiles[tt] = onorm
                qt = qt_tiles[tt]
                psls = psls or [psl]
                credit = 0.0
                nl = 0

                def fill():
                    nonlocal credit
                    credit += rate
                    while credit >= 1.0 and filler is not None:
                        try:
                            next(filler)
                        except StopIteration:
                            break
                        credit -= 1.0

                for h in range(HL):
                    psO = pso.tile([P, TT], f32, tag="pso", name=f"psO{tt}_{h}")
                    # softmax denominator: two accumulators so DVE (even
                    # blocks) and Pool (odd blocks) split the adds
                    esumA = esp.tile([P, TT], f32, tag="esA", name=f"esA{tt}_{h}")
                    esumB = esp.tile([P, TT], f32, tag="esB", name=f"esB{tt}_{h}")
                    prev = None
                    for i, sb in enumerate(range(nsb)):
                        k = sb - 4 * tt
                        c0 = k * P if k >= 0 else 0
                        pslp = psls[nl % len(psls)]
                        nl += 1
                        psL = pslp.tile([P, TT], f32, tag="psl", name="psL")
                        if k >= 0:
                            mm("QK", psL[:, c0:], kt_all[:, h, sb * P:(sb + 1) * P],
                               qt[:, h, c0:], start=True, stop=False)
                            mm("MB", psL[:, c0:c0 + P], ident[:], tri[:],
                               start=False, stop=True)
                        else:
                            mm("QK", psL[:, :], kt_all[:, h, sb * P:(sb + 1) * P],
                               qt[:, h, :])
                        ep = epp.tile([P, TT], bf16, tag="ep", name="ep")
                        with nc.allow_low_precision(reason="softmax probs bf16"):
                            nc.scalar.activation(ep[:, :TT - c0], psL[:, c0:],
                                                 AF.Exp, scale=SCALE2)
                        eng = nc.vector if i % 2 == 0 else nc.gpsimd
                        esum = esumA if i % 2 == 0 else esumB
                        if i == 0:
                            nc.vector.tensor_copy(esumA[:], ep[:])
                        elif i == 1:
                            if c0 > 0:
                                nc.gpsimd.memset(esumB[:, :c0], 0.0)
                            nc.gpsimd.tensor_copy(esumB[:, c0:], ep[:, :TT - c0])
                        else:
                            eng.tensor_add(esum[:, c0:], esum[:, c0:],
                                           ep[:, :TT - c0])
                        drain(1)
                        if prev is not None:
                            psb, pep, pc0 = prev
                            mm("AV", psO[:, pc0:],
                               v_all[:, psb, h * D:(h + 1) * D],
                               pep[:, :TT - pc0],
                               start=(psb == 0), stop=False)
                        fill()
                        prev = (sb, ep, c0)
                    psb, pep, pc0 = prev
                    mm("AV", psO[:, pc0:], v_all[:, psb, h * D:(h + 1) * D],
                       pep[:, :TT - pc0], start=(psb == 0), stop=True)
                    pending.append(tail_gen(tt, h, psO, esumA, esumB, onorm))

            # ---------------- DMA schedule (sync queue order) ----------------
            # phase-A operands stream first (PE start is gated on them);
            # '8' (main) terms before 'r' (residual) terms, matching the
            # accumulation order inside each psum group.
            for c in range(NCH):
                load_w(wq_c, wq_d, [c], "8"); load_xq(0, [c], "8")
            for c in range(NCH):
                load_w(wq_c, wq_d, [c], "r"); load_xq(0, [c], "r")
            load_w(wk_c, wk_d, range(NCH), "8")
            load_xkv(0)
            load_w(wk_c, wk_d, range(NCH), "r")
            load_w(wv_c, wv_d, range(NCH), "8")
            load_w(wv_c, wv_d, range(NCH), "r")
            load_xkv(1)
            # prefetch the remaining kv tiles ahead of the phase-C loads:
            # the sync queue stalls on the xkv ring waits, which is fine --
            # everything behind is needed much later.
            load_xkv(2)
            load_xkv(3)
            load_xkv(4)
            load_xkv(5)
            load_xq(1)
            load_xkv(6)
            load_xkv(7)
            nc.sync.dma_start(wo_all["8"][:], wo_d["8"][:, :, :])
            nc.sync.dma_start(wo_all["r"][:], wo_d["r"][:, :, :])
            load_xq(2)

            # PE p-state warmup group (garbage accumulation, never read)
            psWU = psl.tile([P, TT], f32, tag="psl", name="psWU")
            for i in range(NWU):
                mm("WU", psWU[:, :P], wup[:], wup[:],
                   start=(i == 0), stop=(i == NWU - 1))

            # Phase A: Q projection for tile 0
            for _ in q_work(0, nbanks=4):
                pass

            # phase-B PSUM pools (opened after phase A's psq0 released banks)
            pskt_cm = tc.tile_pool(name="pskt", bufs=2, space="PSUM")
            pskt = pskt_cm.__enter__()
            psv_cm = tc.tile_pool(name="psv", bufs=2, space="PSUM")
            psv = psv_cm.__enter__()

            # B01: st0, st1 emitted directly (no attention to interleave yet)
            for _ in b_work(range(2)):
                pass

            # BC0: attention(0) with remaining KT/V work as fillers
            fil0 = b_work(range(2, NST))
            attention(0, fil0, rate=54.0)
            for _ in fil0:
                drain(1)

            # close phase-B pools, open psy
            psv_cm.__exit__(None, None, None)
            pskt_cm.__exit__(None, None, None)
            xkvp_cm.__exit__(None, None, None)
            wkvp_cm.__exit__(None, None, None)
            psy_cm = tc.tile_pool(name="psy", bufs=2, space="PSUM")
            psy = psy_cm.__enter__()

            # C1: leading Q(1), then attention(1) + fillers Q(2), YO(0)
            for _ in q_work(1):
                drain(1)
            load_xq(3)

            def chain(*gens):
                for g in gens:
                    yield from g

            fil1 = chain(q_work(2), yo_work(0, [psy]))
            attention(1, fil1, rate=7.0)
            for _ in fil1:
                drain(1)
            fil2 = q_work(3)
            attention(2, fil2, rate=2.0)
            for _ in fil2:
                drain(1)
            # tt3: psq banks are free again -> deepen QK run-ahead
            pslb_cm = tc.tile_pool(name="pslb", bufs=2, space="PSUM")
            pslb = pslb_cm.__enter__()
            fil3 = chain(yo_work(1, [psy]), yo_work(2, [psy]))
            attention(3, fil3, rate=2.6, psls=[psl, pslb])
            pslb_cm.__exit__(None, None, None)

            # coda: remaining fillers + tails + YO(3) double-buffered 4-wide
            psyb_cm = tc.tile_pool(name="psyb", bufs=2, space="PSUM")
            psyb = psyb_cm.__enter__()
            for _ in fil3:
                drain(1)
            for _ in yo_work(3, [psy, psyb]):
                drain(1)
            flush_pending()
            psyb_cm.__exit__(None, None, None)

            psy_cm.__exit__(None, None, None)
            for cm in (yp_cm, rbp_cm, esp_cm, epp_cm, onp_cm,
                       qtp_cm, xqp_cm, wqp_cm):
                cm.__exit__(None, None, None)
            pso_cm.__exit__(None, None, None)
            psl_cm.__exit__(None, None, None)

    nc.compile()
    return nc


_NC_CACHE = {}


def _get_nc(key=0):
    if key not in _NC_CACHE:
        _NC_CACHE[key] = build_nc()
    return _NC_CACHE[key]


def kernel(inputs_q, inputs_kv, Wq, Wk, Wv, Wo):
    import ml_dtypes
    bf = ml_dtypes.bfloat16
    f8 = ml_dtypes.float8_e4m3

    inputs_q = np.asarray(inputs_q, dtype=np.float32)
    inputs_kv = np.asarray(inputs_kv, dtype=np.float32)
    Wq = np.asarray(Wq, dtype=np.float32)
    Wk = np.asarray(Wk, dtype=np.float32)
    Wv = np.asarray(Wv, dtype=np.float32)
    Wo = np.asarray(Wo, dtype=np.float32)

    nc = _get_nc()

    def split8(a):
        """fp8 main + residual pair (a ~ a8 + ar)."""
        a8 = a.astype(f8)
        ar = (a - a8.astype(np.float32)).astype(f8)
        return a8, ar

    def pack(a):  # [NE*P, F] -> [P, NE, F]
        return np.ascontiguousarray(
            a.reshape(NE, P, -1).transpose(1, 0, 2))

    def pack_x(x):  # [T, E] -> two fp8 [P, NE, T]
        x8, xr = split8(np.ascontiguousarray(x.T))
        return pack(x8), pack(xr)

    def pack_w(W, h0):  # [E, N, D] -> two fp8 [P, NE, HD], pre-scaled by WS
        Wg = W[:, h0:h0 + HL, :].reshape(E, HD) * WS
        w8, wr = split8(Wg)
        return pack(w8), pack(wr)

    xq_b = [pack_x(inputs_q[b]) for b in range(B)]
    xkv_b = [pack_x(inputs_kv[b]) for b in range(B)]

    in_maps = []
    for c in range(N_CORES):
        b, g = divmod(c, N_CORES // B)
        h0 = g * HL
        wq8, wqr = pack_w(Wq, h0)
        wk8, wkr = pack_w(Wk, h0)
        wv8, wvr = pack_w(Wv, h0)
        wo8, wor = split8(np.ascontiguousarray(
            (Wo[h0:h0 + HL] * WS).transpose(1, 0, 2)))
        in_maps.append({
            "xq8": xq_b[b][0], "xqr": xq_b[b][1],
            "xkv8": xkv_b[b][0], "xkvr": xkv_b[b][1],
            "wq8": wq8, "wqr": wqr,
            "wk8": wk8, "wkr": wkr,
            "wv8": wv8, "wvr": wvr,
            "wo8": np.ascontiguousarray(wo8),
            "wor": np.ascontiguousarray(wor),
        })

    res = bass_utils.run_bass_kernel_spmd(nc, in_maps, core_ids=list(range(N_CORES)))

    out = np.zeros((B, T, E), dtype=np.float32)
    for c in range(N_CORES):
        b = c // (N_CORES // B)
        yc = np.asarray(res.results[c]["y"]).astype(np.float32)  # [P, T//P, E]
        out[b] += yc.transpose(1, 0, 2).reshape(T, E)
    return out
